# revision 1
# baseline (speedup 1.0000x reference)
"""LIF spike scan kernel for Trainium2 (8 NeuronCores, data-parallel).

Reference computation (per element, scanned over t):
    mem = mem * 0.2 * (1 - spk) + x[t]
    spk = (mem > 0.5)

Carry formulation used here (v = mem * (mem <= 0.5), the post-reset membrane):
    m   = (v * 0.2) + x[t]        -> one DVE scalar_tensor_tensor
    spk = relu(sign(m - 0.5))     -> two ACT ops (exact 0/1 in fp32)
    v   = (m <= 0.5) * m          -> one DVE scalar_tensor_tensor

All arithmetic is fp32 and bit-identical to the jax reference: multiplying by
the exact constants {0.0, 1.0, 0.2} commutes with the reference's rounding.

Sharding: x is [T=16, B=64, C=128, H=32, W=32]; the scan is elementwise over
the 8M spatial elements, so each core takes a contiguous 1/8 slice of the
flattened B*C*H*W axis (8 batches per core) viewed as [T, 128, 8192].
"""

import numpy as np

T = 16
SPATIAL = 64 * 128 * 32 * 32  # 8388608
N_CORES = 8
NPC = SPATIAL // N_CORES      # 1048576 elements per core per timestep
P = 128                       # SBUF partitions
Q = NPC // P                  # 8192 free-dim columns per core
F = 2048                      # free-dim tile size
DECAY = 0.2
THRESH = 0.5

_cache = {}

# Set by test harness to request an NTFF trace / HW timing.
TRACE = False


def _build():
    from contextlib import ExitStack

    import concourse.bacc as bacc
    import concourse.tile as tile
    from concourse import mybir

    f32 = mybir.dt.float32
    u8 = mybir.dt.uint8
    Alu = mybir.AluOpType
    Act = mybir.ActivationFunctionType

    nc = bacc.Bacc("TRN2", target_bir_lowering=False, debug=False)
    x_d = nc.dram_tensor("x", [T, P, Q], f32, kind="ExternalInput").ap()
    # Spikes are exactly 0/1, so ship them as uint8 (4x less store traffic)
    # and widen to fp32 on the host.
    o_d = nc.dram_tensor("spk", [T, P, Q], u8, kind="ExternalOutput").ap()

    # Register -THRESH as a const AP (like Bass.__init__ does for 0.0/1.0):
    # written once before the Tile region + barrier, so activation bias
    # reads are untracked and add no per-instruction semaphore wait (the
    # Activation ISA slot only fits one wait).
    _bias = nc.alloc_sbuf_tensor("const-f32-negthresh", [128, 1], f32)
    nc.gpsimd.memset(_bias.ap(), -THRESH)
    nc.const_aps.aps[(f32, -THRESH)] = _bias.ap()
    nc.all_engine_barrier()

    with tile.TileContext(nc) as tc, ExitStack() as ctx:
        xpool = ctx.enter_context(tc.tile_pool(name="xin", bufs=8))
        vpool = ctx.enter_context(tc.tile_pool(name="vre", bufs=3))
        spool = ctx.enter_context(tc.tile_pool(name="sgn", bufs=3))
        opool = ctx.enter_context(tc.tile_pool(name="out", bufs=4))

        for q0 in range(0, Q, F):
            v = None
            for t in range(T):
                xt = xpool.tile([P, F], f32)
                nc.sync.dma_start(xt[:], x_d[t, :, q0 : q0 + F])
                # mem update in place on the freshly-loaded x tile:
                # m = (v * DECAY) + x[t]; at t=0, m = x[0] exactly.
                m = xt
                if v is not None:
                    nc.vector.scalar_tensor_tensor(
                        m[:], v[:], DECAY, xt[:], op0=Alu.mult, op1=Alu.add
                    )
                s = spool.tile([P, F], f32)
                nc.scalar.activation(s[:], m[:], Act.Sign, bias=-THRESH)
                o = opool.tile([P, F], u8)
                nc.scalar.activation(o[:], s[:], Act.Relu)
                nc.sync.dma_start(o_d[t, :, q0 : q0 + F], o[:])
                if t < T - 1:
                    vn = vpool.tile([P, F], f32)
                    nc.vector.scalar_tensor_tensor(
                        vn[:], m[:], THRESH, m[:], op0=Alu.is_le, op1=Alu.mult
                    )
                    v = vn
    nc.compile()
    return nc


def kernel(x: np.ndarray) -> np.ndarray:
    from concourse.bass_utils import run_bass_kernel_spmd

    if "nc" not in _cache:
        _cache["nc"] = _build()
    nc = _cache["nc"]

    x = np.ascontiguousarray(x, dtype=np.float32).reshape(T, N_CORES, NPC)
    in_maps = [
        {"x": np.ascontiguousarray(x[:, i]).reshape(T, P, Q)} for i in range(N_CORES)
    ]
    res = run_bass_kernel_spmd(
        nc, in_maps, core_ids=list(range(N_CORES)), trace=TRACE
    )
    _cache["last_results"] = res
    out = np.stack(
        [np.asarray(r["spk"]).astype(np.float32).reshape(T, NPC) for r in res.results],
        axis=1,
    )
    return out.reshape(T, 64, 128, 32, 32)

